# revision 27
# baseline (speedup 1.0000x reference)
"""CenterLoss Trainium2 kernel.

Reference computation (see problem statement):
    feats  [N=4096, D=96]  = features.reshape(-1, 96)          (float64 in ref)
    label  [N]             = argmax(predicts, axis=-1)          (fp32 argmax)
    dist_n                 = ||feats_n||^2 + ||c_{l_n}||^2 - 2 feats_n . c_{l_n}
                           = ||feats_n - c_{l_n}||^2
    loss = (sum_n clip(dist_n, 1e-12, 1e12) + (N*C - N) * 1e-12) / N
         -- the (C-1)*1e-12 term comes from clip() lifting the masked-out
            zeros of the [N, C] matrix to 1e-12 each.

Only the labeled column of the [N, C] distance matrix survives the mask, so
the kernel never materializes it: per 128-sample tile it
  1. streams predicts [128, 6625] into SBUF (the dominant cost, ~13.6MB/core),
  2. argmax along the free axis with DVE max / max_index,
  3. indirect-DMA gathers centers[label] rows,
  4. squares (features - gathered) on ACT with accum_out giving the
     per-sample squared distance,
  5. clamps, reduces across partitions with a ones-vector matmul.
Each of the 8 cores handles 512 samples; the host sums the 8 partial sums in
float64 and adds the (C-1)*1e-12 clip constant.
"""

import numpy as np

import concourse.bass as bass
import concourse.mybir as mybir
from concourse import bacc
from concourse.bass_utils import run_bass_kernel_spmd
from concourse.tile import TileContext

NUM_CLASSES = 6625
FEAT_DIM = 96
N_CORES = 8
N_TOTAL = 64 * 64          # 4096 samples
NS = N_TOTAL // N_CORES    # 512 samples per core
P = 128                    # partitions
NTILES = NS // P           # 4 tiles of 128 samples per core
CLAMP_MIN = 1e-12
CLAMP_MAX = 1e12

_NC_CACHE = {}


def _build_nc(reps=1, pred_bufs=4, pass1="hier", dma_only=False, fake_gather=False):
    # reps>1 repeats the whole per-core computation; used only by the
    # benchmark harness to measure steady-state per-iteration device time
    # as the slope between rep counts (cancels launch + kernel-tail cost).
    # pass1: engine strategy for the argmax -
    #   "dve"  - InstMax + full-width InstMaxIndex (two 1x passes)
    #   "hier" - hierarchical: one reduce_max pass over [128,53,125] ->
    #            group maxes, tiny max_index picks the winning group, an
    #            indirect DMA gathers each row's 125-wide segment, and a
    #            tiny max_index finds the in-group position. DVE cost drops
    #            from two full passes to one.
    # dma_only: benchmark variant that loads predicts but skips the argmax,
    #   to measure the pure DMA floor.
    nc = bacc.Bacc("TRN2", target_bir_lowering=False)
    feats = nc.dram_tensor(
        "features", [NS, FEAT_DIM], mybir.dt.float32, kind="ExternalInput"
    )
    preds = nc.dram_tensor(
        "predicts", [NS, NUM_CLASSES], mybir.dt.float32, kind="ExternalInput"
    )
    cents = nc.dram_tensor(
        "centers", [NUM_CLASSES, FEAT_DIM], mybir.dt.float32, kind="ExternalInput"
    )
    out = nc.dram_tensor("out", [1, 1], mybir.dt.float32, kind="ExternalOutput")

    with TileContext(nc) as tc:
        with (
            tc.tile_pool(name="pred", bufs=pred_bufs) as pred_pool,
            tc.tile_pool(name="small", bufs=3) as small_pool,
            tc.tile_pool(name="persist", bufs=1) as persist_pool,
            tc.tile_pool(name="psum", bufs=1, space="PSUM") as psum_pool,
        ):
            ones = persist_pool.tile([P, 1], mybir.dt.float32)
            nc.vector.memset(ones[:], 1.0)

            G, SEG = 53, 125  # 53 * 125 == 6625
            preds_flat = preds[:].rearrange("n (g k) -> (n g) k", k=SEG)
            if pass1 in ("hier", "hierb"):
                # rowbase[p, j] = (j*128 + p) * G, as fp32 for ACT bias use
                rowbase_i = persist_pool.tile([P, NTILES], mybir.dt.int32)
                nc.gpsimd.iota(
                    rowbase_i[:],
                    pattern=[[P * G, NTILES]],
                    base=0,
                    channel_multiplier=G,
                )
                rowbase_f = persist_pool.tile([P, NTILES], mybir.dt.float32)
                nc.vector.tensor_copy(rowbase_f[:], rowbase_i[:])

            if pass1 == "hierb" and not dma_only:
                for _rep in range(reps):
                    dacc = persist_pool.tile([P, NTILES], mybir.dt.float32, tag="dacc")
                    offs_all = small_pool.tile([P, NTILES], mybir.dt.int32, tag="offs")
                    rmax8s = []
                    g8fs = []
                    # phase A: per-tile group-max + winning group
                    for j in range(NTILES):
                        rows = slice(j * P, (j + 1) * P)
                        ptile = pred_pool.tile([P, NUM_CLASSES], mybir.dt.float32)
                        nc.sync.dma_start(out=ptile[:], in_=preds[rows, :])
                        gmax = small_pool.tile([P, G], mybir.dt.float32)
                        nc.vector.reduce_max(
                            gmax[:],
                            ptile[:].rearrange("p (g k) -> p g k", k=SEG),
                            axis=mybir.AxisListType.X,
                        )
                        rmax = small_pool.tile([P, 1], mybir.dt.float32)
                        nc.vector.reduce_max(
                            rmax[:], gmax[:], axis=mybir.AxisListType.X
                        )
                        rmax8 = small_pool.tile([P, 8], mybir.dt.float32, tag=f"rmax8_{j}")
                        nc.scalar.activation(
                            rmax8[:],
                            rmax[:].to_broadcast([P, 8]),
                            mybir.ActivationFunctionType.Copy,
                        )
                        g8 = small_pool.tile([P, 8], mybir.dt.uint32)
                        nc.vector.max_index(g8[:], rmax8[:], gmax[:])
                        g8f = small_pool.tile([P, 1], mybir.dt.float32, tag=f"g8f_{j}")
                        nc.vector.tensor_copy(g8f[:], g8[:, 0:1])
                        offsf = small_pool.tile([P, 1], mybir.dt.float32)
                        nc.scalar.activation(
                            offsf[:],
                            g8f[:],
                            mybir.ActivationFunctionType.Identity,
                            bias=rowbase_f[:, j : j + 1],
                        )
                        nc.vector.tensor_copy(offs_all[:, j : j + 1], offsf[:])
                        rmax8s.append(rmax8)
                        g8fs.append(g8f)
                    # phase B: one batched segment gather for all 4 tiles
                    seg_all = small_pool.tile([P, NTILES, SEG], mybir.dt.float32)
                    nc.gpsimd.indirect_dma_start(
                        out=seg_all[:],
                        out_offset=None,
                        in_=preds_flat,
                        in_offset=bass.IndirectOffsetOnAxis(
                            ap=offs_all[:, 0:NTILES], axis=0
                        ),
                    )
                    idx_all = small_pool.tile([P, NTILES], mybir.dt.uint32, tag="idxall")
                    for j in range(NTILES):
                        k8 = small_pool.tile([P, 8], mybir.dt.uint32)
                        nc.vector.max_index(k8[:], rmax8s[j][:], seg_all[:, j, :])
                        k8f = small_pool.tile([P, 1], mybir.dt.float32)
                        nc.vector.tensor_copy(k8f[:], k8[:, 0:1])
                        idxf = small_pool.tile([P, 1], mybir.dt.float32)
                        nc.scalar.activation(
                            idxf[:],
                            g8fs[j][:],
                            mybir.ActivationFunctionType.Identity,
                            scale=float(SEG),
                            bias=k8f[:],
                        )
                        nc.vector.tensor_copy(idx_all[:, j : j + 1], idxf[:])
                    # phase C: one batched centers gather + distances
                    call = small_pool.tile([P, NTILES, FEAT_DIM], mybir.dt.float32)
                    nc.gpsimd.indirect_dma_start(
                        out=call[:],
                        out_offset=None,
                        in_=cents[:],
                        in_offset=bass.IndirectOffsetOnAxis(
                            ap=idx_all[:, 0:NTILES], axis=0
                        ),
                    )
                    fall = small_pool.tile([P, NTILES, FEAT_DIM], mybir.dt.float32)
                    nc.sync.dma_start(
                        out=fall[:],
                        in_=feats[:].rearrange("(j p) d -> p j d", p=P),
                    )
                    dall = small_pool.tile([P, NTILES, FEAT_DIM], mybir.dt.float32)
                    nc.vector.tensor_tensor(
                        out=dall[:],
                        in0=fall[:],
                        in1=call[:],
                        op=mybir.AluOpType.subtract,
                    )
                    for j in range(NTILES):
                        sq = small_pool.tile([P, FEAT_DIM], mybir.dt.float32)
                        nc.scalar.activation(
                            sq[:],
                            dall[:, j, :],
                            mybir.ActivationFunctionType.Square,
                            accum_out=dacc[:, j : j + 1],
                        )
                    _final_reduce(nc, persist_pool, psum_pool, dacc, ones, out)

            for _rep in range(reps if pass1 != "hierb" or dma_only else 0):
                dacc = persist_pool.tile([P, NTILES], mybir.dt.float32, tag="dacc")
                for j in range(NTILES):
                    rows = slice(j * P, (j + 1) * P)

                    if pass1 == "hierg" and not dma_only:
                        # padded to 52 groups x 128 cols
                        ptile = pred_pool.tile([P, 52 * 128], mybir.dt.float32)
                        nc.sync.dma_start(
                            out=ptile[:, 0:NUM_CLASSES], in_=preds[rows, :]
                        )
                        nc.vector.memset(ptile[:, NUM_CLASSES:], -1e30)
                    else:
                        ptile = pred_pool.tile([P, NUM_CLASSES], mybir.dt.float32)
                        nc.sync.dma_start(out=ptile[:], in_=preds[rows, :])

                    if dma_only:
                        # touch a sliver so the load isn't dead
                        nc.vector.reduce_max(
                            dacc[:, j : j + 1], ptile[:, 0:8],
                            axis=mybir.AxisListType.X,
                        )
                        continue

                    if pass1 == "hierg":
                        # 52 groups of 128 columns (padded with -1e30 so class
                        # index == g*128 + k, recoverable with bit ops)
                        GP, SP = 52, 128
                        gmax = small_pool.tile([P, GP], mybir.dt.float32)
                        nc.vector.reduce_max(
                            gmax[:],
                            ptile[:].rearrange("p (g k) -> p g k", k=SP),
                            axis=mybir.AxisListType.X,
                        )
                        rmax = small_pool.tile([P, 1], mybir.dt.float32)
                        nc.vector.reduce_max(
                            rmax[:], gmax[:], axis=mybir.AxisListType.X
                        )
                        rmax8 = small_pool.tile([P, 8], mybir.dt.float32)
                        nc.scalar.activation(
                            rmax8[:],
                            rmax[:].to_broadcast([P, 8]),
                            mybir.ActivationFunctionType.Copy,
                        )
                        g8 = small_pool.tile([P, 8], mybir.dt.uint32)
                        nc.vector.max_index(g8[:], rmax8[:], gmax[:])
                        # gather each partition's winning 128-wide group from
                        # SBUF on GpSimd. ap_gather broadcasts each of the 16
                        # partition indices to the whole 16-partition group, so
                        # partition p's own winning group lands at block p%16;
                        # the row max value only occurs in p's own group (any
                        # duplicate block is that same group, with the value at
                        # the same in-block position), so a value search over
                        # all 16 blocks yields the right in-group position.
                        g16 = small_pool.tile([P, 1], mybir.dt.int16)
                        nc.vector.tensor_copy(g16[:], g8[:, 0:1])
                        blocks = small_pool.tile([P, 16, SP], mybir.dt.float32)
                        nc.gpsimd.ap_gather(
                            out_ap=blocks[:],
                            in_ap=ptile[:].rearrange("p (g k) -> p g k", k=SP),
                            idxs_ap=g16[:],
                            channels=P,
                            num_elems=GP,
                            d=SP,
                            num_idxs=16,
                        )
                        f8 = small_pool.tile([P, 8], mybir.dt.uint32)
                        nc.vector.max_index(
                            f8[:],
                            rmax8[:],
                            blocks[:].rearrange("p a b -> p (a b)"),
                        )
                        # class index = (g << 7) | (found & 127)
                        kmod = small_pool.tile([P, 1], mybir.dt.uint32)
                        nc.vector.tensor_scalar(
                            out=kmod[:],
                            in0=f8[:, 0:1],
                            scalar1=127,
                            scalar2=None,
                            op0=mybir.AluOpType.bitwise_and,
                        )
                        idx8 = small_pool.tile([P, 1], mybir.dt.uint32)
                        nc.vector.tensor_scalar(
                            out=idx8[:],
                            in0=g8[:, 0:1],
                            scalar1=7,
                            scalar2=kmod[:],
                            op0=mybir.AluOpType.logical_shift_left,
                            op1=mybir.AluOpType.bitwise_or,
                        )
                    elif pass1 == "hier":
                        gmax = small_pool.tile([P, G], mybir.dt.float32)
                        nc.vector.reduce_max(
                            gmax[:],
                            ptile[:].rearrange("p (g k) -> p g k", k=SEG),
                            axis=mybir.AxisListType.X,
                        )
                        rmax = small_pool.tile([P, 1], mybir.dt.float32)
                        nc.vector.reduce_max(
                            rmax[:], gmax[:], axis=mybir.AxisListType.X
                        )
                        rmax8 = small_pool.tile([P, 8], mybir.dt.float32)
                        nc.scalar.activation(
                            rmax8[:],
                            rmax[:].to_broadcast([P, 8]),
                            mybir.ActivationFunctionType.Copy,
                        )
                        g8 = small_pool.tile([P, 8], mybir.dt.uint32)
                        nc.vector.max_index(g8[:], rmax8[:], gmax[:])
                        g8f = small_pool.tile([P, 1], mybir.dt.float32)
                        nc.vector.tensor_copy(g8f[:], g8[:, 0:1])
                        # offset into preds_flat: row*G + g
                        offsf = small_pool.tile([P, 1], mybir.dt.float32)
                        nc.scalar.activation(
                            offsf[:],
                            g8f[:],
                            mybir.ActivationFunctionType.Identity,
                            bias=rowbase_f[:, j : j + 1],
                        )
                        offsi = small_pool.tile([P, 1], mybir.dt.int32)
                        nc.vector.tensor_copy(offsi[:], offsf[:])
                        seg = small_pool.tile([P, SEG], mybir.dt.float32)
                        if fake_gather:
                            # benchmark probe: fixed-window read instead of a
                            # per-row indirect gather (wrong results)
                            nc.vector.tensor_copy(seg[:], ptile[:, 0:SEG])
                        else:
                            nc.gpsimd.indirect_dma_start(
                                out=seg[:],
                                out_offset=None,
                                in_=preds_flat,
                                in_offset=bass.IndirectOffsetOnAxis(
                                    ap=offsi[:, 0:1], axis=0
                                ),
                            )
                        k8 = small_pool.tile([P, 8], mybir.dt.uint32)
                        nc.vector.max_index(k8[:], rmax8[:], seg[:])
                        k8f = small_pool.tile([P, 1], mybir.dt.float32)
                        nc.vector.tensor_copy(k8f[:], k8[:, 0:1])
                        # class index: g*SEG + k
                        idxf = small_pool.tile([P, 1], mybir.dt.float32)
                        nc.scalar.activation(
                            idxf[:],
                            g8f[:],
                            mybir.ActivationFunctionType.Identity,
                            scale=float(SEG),
                            bias=k8f[:],
                        )
                        idx8 = small_pool.tile([P, 1], mybir.dt.uint32)
                        nc.vector.tensor_copy(idx8[:], idxf[:])
                    else:
                        max8 = small_pool.tile([P, 8], mybir.dt.float32)
                        idx8 = small_pool.tile([P, 8], mybir.dt.uint32)
                        nc.vector.max(max8[:], ptile[:])
                        nc.vector.max_index(idx8[:], max8[:], ptile[:])

                    ftile = small_pool.tile([P, FEAT_DIM], mybir.dt.float32)
                    nc.sync.dma_start(out=ftile[:], in_=feats[rows, :])

                    ctile = small_pool.tile([P, FEAT_DIM], mybir.dt.float32)
                    if fake_gather:
                        nc.sync.dma_start(out=ctile[:], in_=cents[0:P, :])
                    else:
                        nc.gpsimd.indirect_dma_start(
                            out=ctile[:],
                            out_offset=None,
                            in_=cents[:],
                            in_offset=bass.IndirectOffsetOnAxis(
                                ap=idx8[:, 0:1], axis=0
                            ),
                        )

                    diff = small_pool.tile([P, FEAT_DIM], mybir.dt.float32)
                    nc.vector.tensor_tensor(
                        out=diff[:],
                        in0=ftile[:],
                        in1=ctile[:],
                        op=mybir.AluOpType.subtract,
                    )
                    sq = small_pool.tile([P, FEAT_DIM], mybir.dt.float32)
                    nc.scalar.activation(
                        sq[:],
                        diff[:],
                        mybir.ActivationFunctionType.Square,
                        accum_out=dacc[:, j : j + 1],
                    )

                _final_reduce(nc, persist_pool, psum_pool, dacc, ones, out)

    nc.compile()
    return nc


def _final_reduce(nc, persist_pool, psum_pool, dacc, ones, out):
    dclamp = persist_pool.tile([P, NTILES], mybir.dt.float32, tag="dclamp")
    nc.vector.tensor_scalar(
        out=dclamp[:],
        in0=dacc[:],
        scalar1=CLAMP_MIN,
        scalar2=CLAMP_MAX,
        op0=mybir.AluOpType.max,
        op1=mybir.AluOpType.min,
    )
    dsum = persist_pool.tile([P, 1], mybir.dt.float32, tag="dsum")
    nc.vector.reduce_sum(dsum[:], dclamp[:], axis=mybir.AxisListType.X)

    res_psum = psum_pool.tile([1, 1], mybir.dt.float32)
    nc.tensor.matmul(res_psum[:], lhsT=dsum[:], rhs=ones[:], start=True, stop=True)
    res_sb = persist_pool.tile([1, 1], mybir.dt.float32, tag="res_sb")
    nc.vector.tensor_copy(res_sb[:], res_psum[:])
    nc.sync.dma_start(out=out[:], in_=res_sb[:])


def _get_nc():
    if "nc" not in _NC_CACHE:
        _NC_CACHE["nc"] = _build_nc()
    return _NC_CACHE["nc"]


def kernel(features, predicts, centers):
    features = np.ascontiguousarray(np.asarray(features, dtype=np.float32))
    predicts = np.ascontiguousarray(np.asarray(predicts, dtype=np.float32))
    centers = np.ascontiguousarray(np.asarray(centers, dtype=np.float32))

    feats = features.reshape(N_TOTAL, FEAT_DIM)
    preds = predicts.reshape(N_TOTAL, NUM_CLASSES)

    in_maps = []
    for c in range(N_CORES):
        rows = slice(c * NS, (c + 1) * NS)
        in_maps.append(
            {
                "features": np.ascontiguousarray(feats[rows]),
                "predicts": np.ascontiguousarray(preds[rows]),
                "centers": centers,
            }
        )

    nc = _get_nc()
    res = run_bass_kernel_spmd(nc, in_maps, list(range(N_CORES)))
    partial = np.array(
        [res.results[i]["out"][0, 0] for i in range(N_CORES)], dtype=np.float64
    )
    loss = partial.sum() / N_TOTAL + (NUM_CLASSES - 1) * CLAMP_MIN
    return np.float64(loss)
